# revision 34
# baseline (speedup 1.0000x reference)
"""Masked dot-product attention on 8 Trainium2 NeuronCores.

Full inputs: queries/keys/values [8, 2048, 128] f32, valid_lens [8] i32.
Output: softmax(Q K^T / sqrt(128), masked to valid_lens) @ V, [8, 2048, 128] f32.

Strategy
--------
Keys at positions >= valid_lens[b] are masked to -1e6 by the reference, so
exp() makes their softmax weight exactly 0: only ceil(vl[b]/128) key-chunks
per batch carry information.  Scores are O(6) in magnitude, so softmax needs
no max-subtraction and partial (numerator, denominator) sums over disjoint
key ranges are additive -- work can be split arbitrarily across cores and
recombined on the host.

The device program (identical on all 8 cores, SPMD) is a flat sequence of
"slots": each slot owns one Q^T tile [128d x 1024q] and C chunk iterations.
Per chunk (128 keys):
  S^T  = K_chunk @ Q^T        two matmuls -> PSUM [128k x 1024q]
  P^T  = exp(scale*S^T + bias[k])  ScalarE, bias is a per-partition mask
                                   (0 valid / -100 masked) -> SBUF bf16
  PV  += P^T_j^T @ [V_chunk | 1]   8 matmuls (stationary P^T slice, moving
                                   V+ones [128k x 129]) accumulating in PSUM
The ones-column yields the softmax denominator in PSUM column 128.

The host schedules (batch, query-half, chunk-range) segments into the
slot grid to balance total chunks per core, builds the per-core input
layouts, and sums/normalizes the partial outputs.
"""

import math
from contextlib import ExitStack

import ml_dtypes
import numpy as np

import concourse.bacc as bacc
import concourse.mybir as mybir
import concourse.tile as tile
from concourse.bass_utils import run_bass_kernel_spmd

N_CORES = 8
B, L, D = 8, 2048, 128
CH = 128          # keys per chunk
WQ = 1024         # queries per slot
QT_N = WQ // 128  # PV matmul subtiles per slot
DV = D + 1        # V columns + ones column
SCALE = 1.0 / math.sqrt(D)
MASK_BIAS = -100.0

BF16 = ml_dtypes.bfloat16


# ---------------------------------------------------------------- scheduling

def _try_pack(groups, structure, order, n_cores):
    """Cut groups (id, nchunks) into segments placed into bins of the given
    structure (one bin per (core, slot)).  Returns {(core, slot): (gid,
    chunk_start, nchunks)} or None if the groups don't fit."""
    bins = []  # [capacity, core, slot]
    for s, c in enumerate(structure):
        for core in range(n_cores):
            bins.append([c, core, s])
    placement = {}
    for gid, total in order:
        done = 0
        while done < total:
            rem = total - done
            if not bins:
                return None
            # fill the largest bin while the remainder overflows it; finish
            # in the smallest bin that fits the remainder.
            bins.sort(key=lambda b: b[0])
            if rem >= bins[-1][0]:
                cap, core, s = bins.pop()
            else:
                i = next(i for i, b in enumerate(bins) if b[0] >= rem)
                cap, core, s = bins.pop(i)
            take = min(cap, rem)
            placement[(core, s)] = (gid, done, take)
            done += take
    return placement


def _schedule(valid_lens):
    """Choose a slot structure [C_1..C_S] (identical on every core) and an
    assignment of (batch, query-half) chunk segments to (core, slot)."""
    import random

    nk = [max(1, -(-int(v) // CH)) for v in valid_lens]
    groups = []  # gid -> (b, qh, nchunks)
    for b in range(B):
        for qh in range(L // WQ):
            groups.append((b, qh, nk[b]))
    sizes = [(gid, g[2]) for gid, g in enumerate(groups)]
    t_all = sum(s for _, s in sizes)
    tpc0 = max(1, -(-t_all // N_CORES))
    rng = random.Random(0)

    def partitions(n, max_parts):
        def rec(n, maxval, parts):
            if n == 0:
                yield list(parts)
                return
            if len(parts) == max_parts:
                return
            for v in range(min(n, maxval), 0, -1):
                parts.append(v)
                yield from rec(n - v, v, parts)
                parts.pop()

        yield from rec(n, n, [])

    best = None  # (tpc, n_slots, structure, placement)
    for tpc in range(tpc0, tpc0 + 2 * max(nk) + 2):
        for structure in partitions(tpc, 4):
            orders = [sorted(sizes, key=lambda x: -x[1])]
            for _ in range(80):
                o = sizes[:]
                rng.shuffle(o)
                orders.append(o)
            for order in orders:
                placement = _try_pack(groups, structure, order, N_CORES)
                if placement is not None:
                    cand = (tpc, len(structure), structure, placement)
                    if best is None or cand[:2] < best[:2]:
                        best = cand
                    break  # this structure is feasible; try next structure
        if best is not None and best[0] == tpc:
            break  # nothing with fewer chunks/core exists at this point
    assert best is not None
    _, _, structure, placement = best
    # assign[core][slot] = (b, qh, chunk_start, nchunks) or None
    assign = [[None] * len(structure) for _ in range(N_CORES)]
    for (core, s), (gid, start, n) in placement.items():
        b, qh, _ = groups[gid]
        assign[core][s] = (b, qh, start, n)
    # Small slot first (its input arrives fastest, compute starts earliest),
    # smallest slot last (shortest exposed final tail), largest in between.
    order = sorted(range(len(structure)), key=lambda s: structure[s])
    first, last = order[1:2], order[:1]
    if not first:
        first = []
    mid = sorted(order[2:], key=lambda s: -structure[s])
    order = first + mid + last
    structure = [structure[s] for s in order]
    assign = [[row[s] for s in order] for row in assign]
    return structure, assign


# ------------------------------------------------------------- device program

def _slot_layout(structure):
    """Per-slot combined input layout: [qt | kt | vx] in one bf16 buffer.
    Returns (offsets, total_width): offsets[s] = (base, kt_off, vx_off)."""
    offsets = []
    base = 0
    for C in structure:
        offsets.append((base, base + WQ, base + WQ + C * CH))
        base += WQ + C * (CH + DV)
    return offsets, base


def _build_program(structure):
    S = len(structure)
    T = sum(structure)
    offsets, totw = _slot_layout(structure)
    nc = bacc.Bacc("TRN2", target_bir_lowering=False, debug=False)
    data_d = nc.dram_tensor("data", [128, totw], mybir.dt.bfloat16,
                            kind="ExternalInput").ap()
    bias_d = nc.dram_tensor("bias", [128, T], mybir.dt.float32,
                            kind="ExternalInput").ap()
    out_d = nc.dram_tensor("out", [S * WQ, DV], mybir.dt.float32,
                           kind="ExternalOutput").ap()

    with tile.TileContext(nc) as tc, ExitStack() as ctx:
        data_pool = ctx.enter_context(tc.tile_pool(name="data", bufs=2))
        bias_pool = ctx.enter_context(tc.tile_pool(name="bias", bufs=1))
        st_pool = ctx.enter_context(tc.tile_pool(name="st", bufs=2,
                                                 space="PSUM"))
        pt_pool = ctx.enter_context(tc.tile_pool(name="pt", bufs=3))
        pv_pool = ctx.enter_context(tc.tile_pool(name="pv", bufs=4,
                                                 space="PSUM"))
        out_pool = ctx.enter_context(tc.tile_pool(name="outs", bufs=2))

        slot_g0 = np.cumsum([0] + structure[:-1]).tolist()

        bias_sb = bias_pool.tile([128, T], mybir.dt.float32)
        nc.scalar.dma_start(bias_sb[:], bias_d[:])

        # PE warmup: ~3.4us of back-to-back dummy matmuls during the initial
        # DMA wait flips the HAM clock gate to 2.4 GHz before real work.
        warm_sb = bias_pool.tile([128, 512], mybir.dt.bfloat16)
        nc.gpsimd.memset(warm_sb[:], 0.0)
        warm_ps = st_pool.tile([128, WQ], mybir.dt.float32, tag="st")
        for _ in range(7):
            nc.tensor.matmul(warm_ps[:, 0:512], warm_sb[:, 0:128], warm_sb[:])

        def emit_ph1(s, C):
            """S^T matmuls + exp for every chunk of slot s; returns the
            slot's P^T tile [128, C*WQ] and its data tile."""
            base, kt0, vx0 = offsets[s]
            w = WQ + C * (CH + DV)
            h = WQ + C * CH
            data_sb = data_pool.tile([128, w], mybir.dt.bfloat16, tag="data")
            # qt+kt pieces ride the sync queue in slot order (per-queue DMAs
            # are serial, so the critical slot-0 piece gets the bandwidth
            # first); bias+vx ride the scalar queue.
            if s == 0:
                h0 = WQ + min(C, 2) * CH
                nc.sync.dma_start(data_sb[:, 0:h0], data_d[:, base:base + h0])
                if h > h0:
                    nc.sync.dma_start(data_sb[:, h0:h],
                                      data_d[:, base + h0:base + h])
            elif s == 1:
                # slot 1 is needed soon after slot 0: use the idle gpsimd
                # queue so it doesn't serialize behind slot 0's pieces
                nc.gpsimd.dma_start(data_sb[:, 0:h], data_d[:, base:base + h])
            else:
                nc.sync.dma_start(data_sb[:, 0:h], data_d[:, base:base + h])
            nc.scalar.dma_start(data_sb[:, h:w], data_d[:, base + h:base + w])
            qt_sb = data_sb[:, 0:WQ]
            kt_sb = data_sb[:, kt0 - base:kt0 - base + C * CH]
            vx_sb = data_sb[:, vx0 - base:vx0 - base + C * DV]
            g0 = slot_g0[s]

            pt_sb = pt_pool.tile([128, C * WQ], mybir.dt.bfloat16, tag="pt")
            for c in range(C):
                st = st_pool.tile([128, WQ], mybir.dt.float32)
                lhs_k = kt_sb[:, c * CH:(c + 1) * CH]
                nc.tensor.matmul(st[:, 0:512], lhs_k, qt_sb[:, 0:512])
                nc.tensor.matmul(st[:, 512:WQ], lhs_k, qt_sb[:, 512:WQ])
                nc.scalar.activation(pt_sb[:, c * WQ:(c + 1) * WQ], st[:],
                                     mybir.ActivationFunctionType.Exp,
                                     bias=bias_sb[:, g0 + c:g0 + c + 1],
                                     scale=SCALE)
            return pt_sb, vx_sb

        def emit_ph2(s, C, pt_sb, vx_sb, is_last):
            """PV accumulation for slot s: per output qtile one sequential
            PSUM accumulation group, rotating through four banks.  Output is
            DMA'd in two pieces so the first half overlaps compute."""
            stage = out_pool.tile([128, QT_N * DV], mybir.dt.float32)
            out_ap = out_d[s * WQ:(s + 1) * WQ, :].rearrange(
                "(j p) c -> p j c", j=QT_N)
            stage_ap = stage[:].rearrange("p (j c) -> p j c", j=QT_N)
            extra = []
            if is_last:
                # no more S^T after the final exp: repurpose the st banks as
                # 4 extra accumulators so all 8 qtile groups trail the exps
                # instead of j4-7 waiting on copy-freed pv banks
                ex0 = st_pool.tile([128, WQ], mybir.dt.float32, tag="st")
                ex1 = st_pool.tile([128, WQ], mybir.dt.float32, tag="st")
                extra = [ex0[:, 0:DV], ex0[:, 512:512 + DV],
                         ex1[:, 0:DV], ex1[:, 512:512 + DV]]
            for j in range(QT_N):
                if is_last and j >= QT_N - 4:
                    pv = extra[j - (QT_N - 4)]
                else:
                    pv = pv_pool.tile([128, DV], mybir.dt.float32, tag="pv")
                for c in range(C):
                    nc.tensor.matmul(pv,
                                     pt_sb[:, c * WQ + j * 128:
                                           c * WQ + (j + 1) * 128],
                                     vx_sb[:, c * DV:(c + 1) * DV],
                                     start=(c == 0), stop=(c == C - 1))
                dst = stage[:, j * DV:(j + 1) * DV]
                if is_last and j % 2 == 1:
                    # ScalarE is idle after the final exp: drain copies on
                    # two engines in parallel
                    nc.scalar.copy(dst, pv)
                else:
                    nc.vector.tensor_copy(dst, pv)
                if j == QT_N // 2 - 1:
                    nc.sync.dma_start(out_ap[:, 0:QT_N // 2, :],
                                      stage_ap[:, 0:QT_N // 2, :])
            eng = nc.scalar if is_last else nc.sync
            eng.dma_start(out_ap[:, QT_N // 2:, :],
                          stage_ap[:, QT_N // 2:, :])

        prev = None
        for s, C in enumerate(structure):
            pt_sb, vx_sb = emit_ph1(s, C)
            if prev is not None:
                emit_ph2(*prev, is_last=False)
            prev = (s, C, pt_sb, vx_sb)
        emit_ph2(*prev, is_last=True)
    nc.compile()
    return nc


# ------------------------------------------------------------------- kernel

def _prep_inputs(queries, keys, values, valid_lens, structure, assign):
    T = sum(structure)
    offsets, totw = _slot_layout(structure)
    slot_g0 = np.cumsum([0] + structure[:-1]).tolist()
    karange = np.arange(CH)
    in_maps = []
    for core in range(N_CORES):
        data = np.zeros((128, totw), dtype=BF16)
        bias = np.full((128, T), MASK_BIAS, dtype=np.float32)
        for s, C in enumerate(structure):
            seg = assign[core][s]
            if seg is None:
                continue
            b, qh, cstart, ncr = seg
            base, kt0, vx0 = offsets[s]
            data[:, base:base + WQ] = queries[b, qh * WQ:(qh + 1) * WQ, :].T
            g = slot_g0[s]
            for ci in range(ncr):
                k0 = (cstart + ci) * CH
                data[:, kt0 + ci * CH:kt0 + (ci + 1) * CH] = \
                    keys[b, k0:k0 + CH, :].T
                data[:, vx0 + ci * DV:vx0 + ci * DV + D] = \
                    values[b, k0:k0 + CH, :]
                valid = (k0 + karange) < int(valid_lens[b])
                data[:, vx0 + ci * DV + D] = valid
                bias[:, g + ci] = np.where(valid, 0.0, MASK_BIAS)
        in_maps.append({"data": data, "bias": bias})
    return in_maps


def _gather(results, structure, assign):
    S = len(structure)
    num = np.zeros((B, L, D), dtype=np.float64)
    den = np.zeros((B, L), dtype=np.float64)
    for core in range(N_CORES):
        out = np.asarray(results[core]["out"], dtype=np.float64)
        out = out.reshape(S, WQ, DV)
        for s in range(S):
            seg = assign[core][s]
            if seg is None:
                continue
            b, qh, _, _ = seg
            rows = slice(qh * WQ, (qh + 1) * WQ)
            num[b, rows, :] += out[s, :, :D]
            den[b, rows] += out[s, :, D]
    return (num / den[:, :, None]).astype(np.float32)


def kernel(queries, keys, values, valid_lens):
    queries = np.asarray(queries, dtype=np.float32)
    keys = np.asarray(keys, dtype=np.float32)
    values = np.asarray(values, dtype=np.float32)
    valid_lens = np.asarray(valid_lens, dtype=np.int32)

    structure, assign = _schedule(valid_lens)
    nc = _build_program(structure)
    in_maps = _prep_inputs(queries, keys, values, valid_lens, structure, assign)
    res = run_bass_kernel_spmd(nc, in_maps, core_ids=list(range(N_CORES)))
    return _gather(res.results, structure, assign)
